# revision 16
# baseline (speedup 1.0000x reference)
"""Trainium2 Bass kernel for nn_Action_PureGRU.

Reference computation (see problem):
    emb = action @ W_emb                       # [B, F, S]
    per step t: parts = [x_t, h] @ W_gru       # [B, 3S]
                LN(parts) (gamma/beta), gates, h update
    h_last, outputs = h, h @ W_dec + b_dec

Strategy: data-parallel over batch across 8 NeuronCores (B=2048 -> 256/core,
processed as two 128-partition tiles). Weights replicated. The input path is
algebraically folded: action @ (W_emb @ W_gru[:S]) so each step's matmul is a
K=1026 contraction (8 k-tiles of h + 1 tiny K=2 tile of the raw action) done
on the PE with fp32 PSUM accumulation. LayerNorm statistics come from chunked
bn_stats on DVE (overlapped with the matmuls); the normalization is folded
into the ScalarE sigmoid/tanh activations via per-partition scale/bias.
1/sqrt(var+eps) is computed on DVE with the int32 bit-trick + 2 Newton steps
(avoids an ACT table swap: sqrt is not in the sigmoid/tanh table set).
h is kept fp32. For the next step's stationary matmul operand h must be
transposed (contraction dim on partitions):
  - mm="bf16": h cast to bf16, one DMA xbar block-transpose per batch tile.
  - mm="f32r": PE transpose (fp32, exact) + ScalarE copy PSUM->SBUF; matmul
    operands bitcast to float32r (full-rate at N>=256, higher precision
    than bf16).
"""

import os
from contextlib import ExitStack

import numpy as np

import concourse.bass as bass
import concourse.bacc as bacc
import concourse.tile as tile
from concourse import mybir
from concourse.bass_utils import run_bass_kernel_spmd
from concourse.masks import make_identity

# Problem constants (hardcoded per harness contract).
B, F, INP, S = 2048, 50, 2, 1024
S3 = 3 * S
LN_EPS = 1e-3
UPDATE_BIAS = -1.0
NCORES = 8
BL = B // NCORES          # 256 rows per core
NBT = BL // 128           # 2 batch tiles of 128 per core
KT = S // 128             # 8 k-tiles of h
NG = 3                    # three 1024-wide gate blocks
F32 = mybir.dt.float32
F32R = mybir.dt.float32r
BF16 = mybir.dt.bfloat16
I32 = mybir.dt.int32

# Variant: mm dtype for the recurrent matmul, dtype for gate tensors.
# f32r matmuls (full PE rate at N>=256, ~12-bit mantissa) + fp32 gates
# measured 1.0e-3 rel err over 50 steps on hardware.
DEFAULT_VARIANT = os.environ.get("GRU_VARIANT", "f32r:f32")

_CACHE: dict = {}
LAST_RESULT = None  # BassKernelResults of the most recent run (for test.py)


def _build_nc(fast_ln: bool, variant: str = DEFAULT_VARIANT,
              n_steps: int = F, n_btiles: int = NBT):
    """Build + compile the per-core Bass program (same program on all cores)."""
    mm, gd = variant.split(":")
    assert mm in ("bf16", "f32r") and gd in ("bf16", "f32")
    MMD = BF16 if mm == "bf16" else F32R  # storage dtype of W_h / W_dec / hT
    GD = BF16 if gd == "bf16" else F32    # gate tensor dtype

    def mmview(ap):  # matmul-operand view of an MMD tensor
        return ap

    nc = bacc.Bacc("TRN2", target_bir_lowering=False, debug=False)

    # ---- DRAM I/O (per core) ----
    bl = n_btiles * 128
    d_at = nc.dram_tensor("a_t", [2, n_steps, bl], MMD, kind="ExternalInput").ap()
    d_wh = nc.dram_tensor("w_h", [128, KT, S3], MMD, kind="ExternalInput").ap()
    d_wc = nc.dram_tensor("w_c", [2, S3], MMD, kind="ExternalInput").ap()
    d_wd = nc.dram_tensor("w_d", [128, KT, INP], MMD, kind="ExternalInput").ap()
    d_bd = nc.dram_tensor("b_d", [128, INP], F32, kind="ExternalInput").ap()
    if not fast_ln:
        d_gm = nc.dram_tensor("gm", [1, S3], F32, kind="ExternalInput").ap()
        d_bt = nc.dram_tensor("bt", [1, S3], F32, kind="ExternalInput").ap()
    d_h = nc.dram_tensor("h_out", [bl, S], F32, kind="ExternalOutput").ap()
    d_y = nc.dram_tensor("y_out", [bl, INP], F32, kind="ExternalOutput").ap()

    N_NEWTON = int(os.environ.get("GRU_NEWTON", "2"))
    MAGIC = 0x5F3759DF + 1  # rsqrt seed: C+1 (the +1 folds two's-complement negate)

    with tile.TileContext(nc) as tc, ExitStack() as ctx:
        singles = ctx.enter_context(tc.tile_pool(name="singles", bufs=1))
        psum = ctx.enter_context(tc.tile_pool(name="psum", bufs=4, space="PSUM"))
        gates = ctx.enter_context(tc.tile_pool(name="gates", bufs=2))
        stats = ctx.enter_context(tc.tile_pool(name="stats", bufs=4))

        # Resident weights / inputs.
        wh_sb = singles.tile([128, KT + 1, S3], MMD)
        nc.vector.memset(wh_sb[:, KT, :].bitcast(F32), 0.0)
        nc.sync.dma_start(out=wh_sb[:, :KT, :], in_=d_wh)
        nc.sync.dma_start(out=wh_sb[0:2, KT, :], in_=d_wc)
        wd_sb = singles.tile([128, KT, INP], MMD)
        nc.sync.dma_start(out=wd_sb, in_=d_wd)
        bd_sb = singles.tile([128, INP], F32)
        nc.sync.dma_start(out=bd_sb, in_=d_bd)
        if not fast_ln:
            gm_sb = singles.tile([128, S3], F32)
            nc.sync.dma_start(out=gm_sb, in_=d_gm.to_broadcast((128, S3)))
            bt_sb = singles.tile([128, S3], F32)
            nc.sync.dma_start(out=bt_sb, in_=d_bt.to_broadcast((128, S3)))
        if mm == "f32r":
            ident = singles.tile([128, 128], F32)
            make_identity(nc, ident)

        # Persistent per-btile state.
        h_sb = [singles.tile([128, S], F32, tag=f"h{m}", name=f"h{m}")
                for m in range(n_btiles)]
        hT = [singles.tile([128, KT + 1, 128], MMD, tag=f"hT{m}", name=f"hT{m}")
              for m in range(n_btiles)]
        for m in range(n_btiles):
            nc.vector.memset(hT[m][:, KT, :].bitcast(F32) if mm == "f32r"
                             else hT[m][:, KT, :], 0.0)
        hb_tiles = [None] * n_btiles

        def make_hT(m, hb):
            """hT[m][:, k, :] = h[:, k*128:(k+1)*128].T  (hb = bf16 copy or None)"""
            if mm == "bf16":
                eng = nc.sync if m % 2 == 0 else nc.scalar
                eng.dma_start_transpose(hT[m][:, :KT, :], hb)
            else:
                tp = psum.tile([128, 1024], F32, tag="parts", name="tp")
                for k in range(KT):
                    nc.tensor.transpose(
                        tp[:, k * 128:(k + 1) * 128], h_sb[m][:, k * 128:(k + 1) * 128],
                        ident,
                    )
                nc.scalar.copy(hT[m][:, :KT // 2, :], tp[:, :512])
                nc.vector.tensor_copy(hT[m][:, KT // 2:KT, :], tp[:, 512:])

        tr_pend = [False] * n_btiles

        def emit_tail(m, t):
            """LN stats + gates + state update for (t, m); halved passes."""
            st6, mv, ps = tail_state[m]
            nc.vector.bn_aggr(out=mv, in_=st6)
            vpe = stats.tile([128, 1], F32, tag="vpe", name="vpe")
            nc.vector.tensor_scalar_add(vpe, mv[:, 1:2], LN_EPS)
            yi = stats.tile([128, 1], I32, tag="yi", name="yi")
            nc.vector.tensor_scalar(
                out=yi,
                in0=vpe.bitcast(I32),
                scalar1=1,
                scalar2=-1,
                op0=mybir.AluOpType.logical_shift_right,
                op1=mybir.AluOpType.bitwise_xor,
            )
            nc.vector.tensor_scalar_add(yi, yi, MAGIC)
            y = yi.bitcast(F32)
            for it in range(N_NEWTON):
                t1 = stats.tile([128, 1], F32, tag="nt1", name="nt1")
                nc.vector.tensor_mul(t1, y, y)
                nc.vector.scalar_tensor_tensor(
                    out=t1, in0=t1, scalar=-0.5, in1=vpe,
                    op0=mybir.AluOpType.mult, op1=mybir.AluOpType.mult,
                )
                yn = stats.tile([128, 1], F32, tag="yn", name="yn")
                nc.vector.scalar_tensor_tensor(
                    out=yn, in0=t1, scalar=1.5, in1=y,
                    op0=mybir.AluOpType.add, op1=mybir.AluOpType.mult,
                )
                y = yn
            inv_std = y
            s2 = stats.tile([128, 1], F32, tag="s2", name="s2")
            nc.vector.scalar_tensor_tensor(
                out=s2, in0=mv[:, 0:1], scalar=-1.0, in1=inv_std,
                op0=mybir.AluOpType.mult, op1=mybir.AluOpType.mult,
            )
            s2u = stats.tile([128, 1], F32, tag="s2u", name="s2u")
            nc.vector.tensor_scalar_add(s2u, s2, UPDATE_BIAS)

            if fast_ln:
                zr, zc, zu = ps[0][:], ps[1][:], ps[2][:]
                r = gates.tile([128, S], GD, tag="rd", name="r")
                u = gates.tile([128, S], GD, tag="u", name="u")
                tt = gates.tile([128, S], GD, tag="te", name="tt")
                c = gates.tile([128, S], GD, tag="c", name="c")
                if t > 0:
                    d = gates.tile([128, S], GD, tag="d", name="d")
                    e = gates.tile([128, S], GD, tag="te2", name="e")
                if mm == "bf16":
                    hb = gates.tile([128, S], BF16, tag="hb", name="hb")
                    hb_tiles[m] = hb
                nc.scalar.activation(
                    r, zr, mybir.ActivationFunctionType.Sigmoid,
                    bias=s2, scale=inv_std,
                )
                nc.scalar.activation(
                    u, zu, mybir.ActivationFunctionType.Sigmoid,
                    bias=s2u, scale=inv_std,
                )
                nc.vector.scalar_tensor_tensor(
                    out=tt, in0=zc, scalar=mv[:, 0:1], in1=r,
                    op0=mybir.AluOpType.subtract, op1=mybir.AluOpType.mult,
                )
                nc.scalar.activation(
                    c, tt, mybir.ActivationFunctionType.Tanh, scale=inv_std,
                )
                if t == 0:
                    nc.vector.tensor_mul(h_sb[m], u, c)
                else:
                    nc.vector.tensor_sub(d, c, h_sb[m])
                    nc.vector.tensor_mul(e, u, d)
                    nc.vector.tensor_add(h_sb[m], h_sb[m], e)
                if mm == "bf16":
                    nc.vector.tensor_copy(hb, h_sb[m])
            else:
                zs = []
                for g in range(NG):
                    zg = gates.tile([128, 1024], F32, tag=f"z{g}", name="zg")
                    nc.vector.tensor_scalar(
                        out=zg, in0=ps[g][:], scalar1=mv[:, 0:1],
                        scalar2=inv_std,
                        op0=mybir.AluOpType.subtract,
                        op1=mybir.AluOpType.mult,
                    )
                    gsl = slice(g * 1024, (g + 1) * 1024)
                    nc.vector.tensor_mul(zg, zg, gm_sb[:, gsl])
                    nc.vector.tensor_add(zg, zg, bt_sb[:, gsl])
                    zs.append(zg)
                r = gates.tile([128, S], GD, tag="rd", name="r")
                nc.scalar.activation(
                    r, zs[0][:], mybir.ActivationFunctionType.Sigmoid)
                u = gates.tile([128, S], GD, tag="u", name="u")
                nc.scalar.activation(
                    u, zs[2][:], mybir.ActivationFunctionType.Sigmoid,
                    bias=UPDATE_BIAS, scale=1.0,
                )
                tt = gates.tile([128, S], GD, tag="te", name="tt")
                nc.vector.tensor_mul(tt, zs[1][:], r)
                c = gates.tile([128, S], GD, tag="c", name="c")
                nc.scalar.activation(c, tt, mybir.ActivationFunctionType.Tanh)
                if t == 0:
                    nc.vector.tensor_mul(h_sb[m], u, c)
                else:
                    d = gates.tile([128, S], GD, tag="d", name="d")
                    nc.vector.tensor_sub(d, c, h_sb[m])
                    e = gates.tile([128, S], GD, tag="te2", name="e")
                    nc.vector.tensor_mul(e, u, d)
                    nc.vector.tensor_add(h_sb[m], h_sb[m], e)
                if mm == "bf16":
                    hb = gates.tile([128, S], BF16, tag="hb", name="hb")
                    nc.vector.tensor_copy(hb, h_sb[m])
                    hb_tiles[m] = hb
            tr_pend[m] = True

        tail_state = [None] * n_btiles
        for t in range(n_steps):
            for m in range(n_btiles):
                other = (m + 1) % n_btiles
                msl = slice(m * 128, (m + 1) * 128)
                nc.sync.dma_start(out=hT[m][0:2, KT, :], in_=d_at[:, t, msl])
                ps = []
                st6 = stats.tile([128, 2 * NG, 6], F32, tag="bnst", name="st6")
                if tr_pend[m]:
                    make_hT(m, hb_tiles[m])
                    tr_pend[m] = False
                for g in range(NG):
                    pt = psum.tile([128, 1024], F32, tag="parts", name="pt")
                    ps.append(pt)
                    for j in range(2):
                        nsl = slice(g * 1024 + j * 512, g * 1024 + (j + 1) * 512)
                        jsl = slice(j * 512, (j + 1) * 512)
                        kts = [KT] if t == 0 else [KT] + list(range(KT))
                        for i, k in enumerate(kts):
                            nc.tensor.matmul(
                                pt[:, jsl],
                                mmview(hT[m][:, k, :]),
                                mmview(wh_sb[:, k, nsl]),
                                start=(i == 0),
                                stop=(i == len(kts) - 1),
                            )
                        nc.vector.bn_stats(out=st6[:, 2 * g + j, :], in_=pt[:, jsl])
                tail_state[m] = (st6, stats.tile([128, 2], F32, tag="mv",
                                                 name="mv"), ps)
                emit_tail(m, t)

        # epilogue: h_last + decoder
        for m in range(n_btiles):
            msl = slice(m * 128, (m + 1) * 128)
            if tr_pend[m]:
                make_hT(m, hb_tiles[m])
                tr_pend[m] = False
            nc.sync.dma_start(out=d_h[msl, :], in_=h_sb[m])
            py = psum.tile([128, INP], F32, tag="parts", name="py")
            for k in range(KT):
                nc.tensor.matmul(
                    py,
                    mmview(hT[m][:, k, :]),
                    mmview(wd_sb[:, k, :]),
                    start=(k == 0),
                    stop=(k == KT - 1),
                )
            ysb = gates.tile([128, INP], F32, tag="y", name="ysb")
            nc.vector.tensor_add(ysb, py, bd_sb)
            nc.sync.dma_start(out=d_y[msl, :], in_=ysb)

    nc.compile()
    return nc


def _prep_inputs(action, W_emb, W_gru, ln_gamma, ln_beta, W_dec, b_dec,
                 fast_ln, variant=DEFAULT_VARIANT, n_steps=F, n_btiles=NBT):
    """Host-side weight folding + per-core shard dicts."""
    import ml_dtypes

    mm, _ = variant.split(":")
    bf = ml_dtypes.bfloat16
    mmdt = bf if mm == "bf16" else np.float32
    W_comb = (W_emb.astype(np.float64) @ W_gru[:S].astype(np.float64))
    wc = np.ascontiguousarray(W_comb.astype(np.float32).astype(mmdt))      # [2, 3S]
    wh = np.ascontiguousarray(
        W_gru[S:].astype(np.float32).reshape(KT, 128, S3).transpose(1, 0, 2)
        .astype(mmdt))                                                     # [128, KT, 3S]
    wd = np.ascontiguousarray(
        W_dec.astype(np.float32).reshape(KT, 128, INP).transpose(1, 0, 2)
        .astype(mmdt))                                                     # [128, KT, 2]
    bd = np.ascontiguousarray(
        np.broadcast_to(b_dec.astype(np.float32), (128, INP)))             # [128, 2]

    bl = n_btiles * 128
    in_maps = []
    for cix in range(NCORES):
        a_c = action[cix * BL: cix * BL + bl, :n_steps, :]                 # [bl, t, 2]
        at = np.ascontiguousarray(a_c.transpose(2, 1, 0).astype(mmdt))     # [2, t, bl]
        m = {"a_t": at, "w_h": wh, "w_c": wc, "w_d": wd, "b_d": bd}
        if not fast_ln:
            m["gm"] = np.ascontiguousarray(ln_gamma.astype(np.float32))[None, :]
            m["bt"] = np.ascontiguousarray(ln_beta.astype(np.float32))[None, :]
        in_maps.append(m)
    return in_maps


def kernel(action, W_emb, W_gru, ln_gamma, ln_beta, W_dec, b_dec):
    global LAST_RESULT
    action = np.asarray(action)
    W_emb = np.asarray(W_emb)
    W_gru = np.asarray(W_gru)
    ln_gamma = np.asarray(ln_gamma)
    ln_beta = np.asarray(ln_beta)
    W_dec = np.asarray(W_dec)
    b_dec = np.asarray(b_dec)

    fast_ln = bool(np.all(ln_gamma == 1.0) and np.all(ln_beta == 0.0))
    variant = DEFAULT_VARIANT
    key = ("nc", fast_ln, variant)
    if key not in _CACHE:
        _CACHE[key] = _build_nc(fast_ln, variant)
    nc = _CACHE[key]

    in_maps = _prep_inputs(
        action, W_emb, W_gru, ln_gamma, ln_beta, W_dec, b_dec, fast_ln, variant)

    trace = bool(os.environ.get("GRU_TRACE"))
    res = run_bass_kernel_spmd(
        nc, in_maps, core_ids=list(range(NCORES)), trace=trace)
    LAST_RESULT = res

    h_full = np.concatenate(
        [np.asarray(res.results[c]["h_out"]) for c in range(NCORES)], axis=0)
    y_full = np.concatenate(
        [np.asarray(res.results[c]["y_out"]) for c in range(NCORES)], axis=0)
    return h_full[:, None, :].astype(np.float32), y_full[:, None, :].astype(np.float32)


# revision 17
# speedup vs baseline: 1.1950x; 1.1950x over previous
"""Trainium2 Bass kernel for nn_Action_PureGRU.

Reference computation (see problem):
    emb = action @ W_emb                       # [B, F, S]
    per step t: parts = [x_t, h] @ W_gru       # [B, 3S]
                LN(parts) (gamma/beta), gates, h update
    h_last, outputs = h, h @ W_dec + b_dec

Strategy: data-parallel over batch across 8 NeuronCores (B=2048 -> 256/core,
processed as two 128-partition tiles). Weights replicated. The input path is
algebraically folded: action @ (W_emb @ W_gru[:S]) so each step's matmul is a
K=1026 contraction (8 k-tiles of h + 1 tiny K=2 tile of the raw action) done
on the PE with fp32 PSUM accumulation. LayerNorm statistics come from chunked
bn_stats on DVE (overlapped with the matmuls); the normalization is folded
into the ScalarE sigmoid/tanh activations via per-partition scale/bias.
1/sqrt(var+eps) is computed on DVE with the int32 bit-trick + 2 Newton steps
(avoids an ACT table swap: sqrt is not in the sigmoid/tanh table set).
h is kept fp32. For the next step's stationary matmul operand h must be
transposed (contraction dim on partitions):
  - mm="bf16": h cast to bf16, one DMA xbar block-transpose per batch tile.
  - mm="f32r": PE transpose (fp32, exact) + ScalarE copy PSUM->SBUF; matmul
    operands bitcast to float32r (full-rate at N>=256, higher precision
    than bf16).
"""

import os
from contextlib import ExitStack

import numpy as np

import concourse.bass as bass
import concourse.bacc as bacc
import concourse.tile as tile
from concourse import mybir
from concourse.bass_utils import run_bass_kernel_spmd
from concourse.masks import make_identity

# Problem constants (hardcoded per harness contract).
B, F, INP, S = 2048, 50, 2, 1024
S3 = 3 * S
LN_EPS = 1e-3
UPDATE_BIAS = -1.0
NCORES = 8
BL = B // NCORES          # 256 rows per core
NBT = BL // 128           # 2 batch tiles of 128 per core
KT = S // 128             # 8 k-tiles of h
NG = 3                    # three 1024-wide gate blocks
F32 = mybir.dt.float32
F32R = mybir.dt.float32r
BF16 = mybir.dt.bfloat16
I32 = mybir.dt.int32

# Variant: mm dtype for the recurrent matmul, dtype for gate tensors.
# f32r matmuls (full PE rate at N>=256, ~12-bit mantissa) + fp32 gates
# measured 1.0e-3 rel err over 50 steps on hardware.
DEFAULT_VARIANT = os.environ.get("GRU_VARIANT", "f32r:f32")

_CACHE: dict = {}
LAST_RESULT = None  # BassKernelResults of the most recent run (for test.py)


def _build_nc(fast_ln: bool, variant: str = DEFAULT_VARIANT,
              n_steps: int = F, n_btiles: int = NBT):
    """Build + compile the per-core Bass program (same program on all cores)."""
    mm, gd = variant.split(":")
    assert mm in ("bf16", "f32r") and gd in ("bf16", "f32")
    MMD = BF16 if mm == "bf16" else F32R  # storage dtype of W_h / W_dec / hT
    GD = BF16 if gd == "bf16" else F32    # gate tensor dtype

    def mmview(ap):  # matmul-operand view of an MMD tensor
        return ap

    nc = bacc.Bacc("TRN2", target_bir_lowering=False, debug=False)

    # ---- DRAM I/O (per core) ----
    bl = n_btiles * 128
    d_at = nc.dram_tensor("a_t", [2, n_steps, bl], MMD, kind="ExternalInput").ap()
    d_wh = nc.dram_tensor("w_h", [128, KT, S3], MMD, kind="ExternalInput").ap()
    d_wc = nc.dram_tensor("w_c", [2, S3], MMD, kind="ExternalInput").ap()
    d_wd = nc.dram_tensor("w_d", [128, KT, INP], MMD, kind="ExternalInput").ap()
    d_bd = nc.dram_tensor("b_d", [128, INP], F32, kind="ExternalInput").ap()
    if not fast_ln:
        d_gm = nc.dram_tensor("gm", [1, S3], F32, kind="ExternalInput").ap()
        d_bt = nc.dram_tensor("bt", [1, S3], F32, kind="ExternalInput").ap()
    d_h = nc.dram_tensor("h_out", [bl, S], F32, kind="ExternalOutput").ap()
    d_y = nc.dram_tensor("y_out", [bl, INP], F32, kind="ExternalOutput").ap()

    N_NEWTON = int(os.environ.get("GRU_NEWTON", "2"))
    MAGIC = 0x5F3759DF + 1  # rsqrt seed: C+1 (the +1 folds two's-complement negate)

    with tile.TileContext(nc) as tc, ExitStack() as ctx:
        singles = ctx.enter_context(tc.tile_pool(name="singles", bufs=1))
        psum = ctx.enter_context(tc.tile_pool(name="psum", bufs=4, space="PSUM"))
        gates = ctx.enter_context(tc.tile_pool(name="gates", bufs=2))
        stats = ctx.enter_context(tc.tile_pool(name="stats", bufs=4))

        # Resident weights / inputs.
        wh_sb = singles.tile([128, KT + 1, S3], MMD)
        nc.vector.memset(wh_sb[:, KT, :].bitcast(F32), 0.0)
        nc.sync.dma_start(out=wh_sb[:, :KT, :], in_=d_wh)
        nc.sync.dma_start(out=wh_sb[0:2, KT, :], in_=d_wc)
        wd_sb = singles.tile([128, KT, INP], MMD)
        nc.sync.dma_start(out=wd_sb, in_=d_wd)
        bd_sb = singles.tile([128, INP], F32)
        nc.sync.dma_start(out=bd_sb, in_=d_bd)
        if not fast_ln:
            gm_sb = singles.tile([128, S3], F32)
            nc.sync.dma_start(out=gm_sb, in_=d_gm.to_broadcast((128, S3)))
            bt_sb = singles.tile([128, S3], F32)
            nc.sync.dma_start(out=bt_sb, in_=d_bt.to_broadcast((128, S3)))
        if mm == "f32r":
            ident = singles.tile([128, 128], F32)
            make_identity(nc, ident)

        # Persistent per-btile state.
        h_sb = [singles.tile([128, S], F32, tag=f"h{m}", name=f"h{m}")
                for m in range(n_btiles)]
        hT = [singles.tile([128, KT + 1, 128], MMD, tag=f"hT{m}", name=f"hT{m}")
              for m in range(n_btiles)]
        for m in range(n_btiles):
            nc.vector.memset(hT[m][:, KT, :].bitcast(F32) if mm == "f32r"
                             else hT[m][:, KT, :], 0.0)
        hb_tiles = [None] * n_btiles

        def make_hT(m, hb):
            """hT[m][:, k, :] = h[:, k*128:(k+1)*128].T  (hb = bf16 copy or None)"""
            if mm == "bf16":
                eng = nc.sync if m % 2 == 0 else nc.scalar
                eng.dma_start_transpose(hT[m][:, :KT, :], hb)
            else:
                tp = psum.tile([128, 1024], F32, tag="parts", name="tp")
                for k in range(KT):
                    nc.tensor.transpose(
                        tp[:, k * 128:(k + 1) * 128], h_sb[m][:, k * 128:(k + 1) * 128],
                        ident,
                    )
                nc.scalar.copy(hT[m][:, :KT, :], tp)

        tr_pend = [False] * n_btiles

        def emit_tail(m, t):
            """LN stats + gates + state update for (t, m); halved passes."""
            st6, mv, ps = tail_state[m]
            nc.vector.bn_aggr(out=mv, in_=st6)
            vpe = stats.tile([128, 1], F32, tag="vpe", name="vpe")
            nc.vector.tensor_scalar_add(vpe, mv[:, 1:2], LN_EPS)
            yi = stats.tile([128, 1], I32, tag="yi", name="yi")
            nc.vector.tensor_scalar(
                out=yi,
                in0=vpe.bitcast(I32),
                scalar1=1,
                scalar2=-1,
                op0=mybir.AluOpType.logical_shift_right,
                op1=mybir.AluOpType.bitwise_xor,
            )
            nc.vector.tensor_scalar_add(yi, yi, MAGIC)
            y = yi.bitcast(F32)
            for it in range(N_NEWTON):
                t1 = stats.tile([128, 1], F32, tag="nt1", name="nt1")
                nc.vector.tensor_mul(t1, y, y)
                nc.vector.scalar_tensor_tensor(
                    out=t1, in0=t1, scalar=-0.5, in1=vpe,
                    op0=mybir.AluOpType.mult, op1=mybir.AluOpType.mult,
                )
                yn = stats.tile([128, 1], F32, tag="yn", name="yn")
                nc.vector.scalar_tensor_tensor(
                    out=yn, in0=t1, scalar=1.5, in1=y,
                    op0=mybir.AluOpType.add, op1=mybir.AluOpType.mult,
                )
                y = yn
            inv_std = y
            s2 = stats.tile([128, 1], F32, tag="s2", name="s2")
            nc.vector.scalar_tensor_tensor(
                out=s2, in0=mv[:, 0:1], scalar=-1.0, in1=inv_std,
                op0=mybir.AluOpType.mult, op1=mybir.AluOpType.mult,
            )
            s2u = stats.tile([128, 1], F32, tag="s2u", name="s2u")
            nc.vector.tensor_scalar_add(s2u, s2, UPDATE_BIAS)

            if fast_ln:
                zr, zc, zu = ps[0][:], ps[1][:], ps[2][:]
                r = gates.tile([128, S], GD, tag="rd", name="r")
                u = gates.tile([128, S], GD, tag="u", name="u")
                tt = gates.tile([128, S], GD, tag="te", name="tt")
                c = gates.tile([128, S], GD, tag="c", name="c")
                if t > 0:
                    d = gates.tile([128, S], GD, tag="d", name="d")
                    e = gates.tile([128, S], GD, tag="te2", name="e")
                if mm == "bf16":
                    hb = gates.tile([128, S], BF16, tag="hb", name="hb")
                    hb_tiles[m] = hb
                nc.scalar.activation(
                    r, zr, mybir.ActivationFunctionType.Sigmoid,
                    bias=s2, scale=inv_std,
                )
                nc.scalar.activation(
                    u, zu, mybir.ActivationFunctionType.Sigmoid,
                    bias=s2u, scale=inv_std,
                )
                nc.vector.scalar_tensor_tensor(
                    out=tt, in0=zc, scalar=mv[:, 0:1], in1=r,
                    op0=mybir.AluOpType.subtract, op1=mybir.AluOpType.mult,
                )
                nc.scalar.activation(
                    c, tt, mybir.ActivationFunctionType.Tanh, scale=inv_std,
                )
                if t == 0:
                    nc.vector.tensor_mul(h_sb[m], u, c)
                else:
                    nc.vector.tensor_sub(d, c, h_sb[m])
                    nc.vector.tensor_mul(e, u, d)
                    nc.vector.tensor_add(h_sb[m], h_sb[m], e)
                if mm == "bf16":
                    nc.vector.tensor_copy(hb, h_sb[m])
            else:
                zs = []
                for g in range(NG):
                    zg = gates.tile([128, 1024], F32, tag=f"z{g}", name="zg")
                    nc.vector.tensor_scalar(
                        out=zg, in0=ps[g][:], scalar1=mv[:, 0:1],
                        scalar2=inv_std,
                        op0=mybir.AluOpType.subtract,
                        op1=mybir.AluOpType.mult,
                    )
                    gsl = slice(g * 1024, (g + 1) * 1024)
                    nc.vector.tensor_mul(zg, zg, gm_sb[:, gsl])
                    nc.vector.tensor_add(zg, zg, bt_sb[:, gsl])
                    zs.append(zg)
                r = gates.tile([128, S], GD, tag="rd", name="r")
                nc.scalar.activation(
                    r, zs[0][:], mybir.ActivationFunctionType.Sigmoid)
                u = gates.tile([128, S], GD, tag="u", name="u")
                nc.scalar.activation(
                    u, zs[2][:], mybir.ActivationFunctionType.Sigmoid,
                    bias=UPDATE_BIAS, scale=1.0,
                )
                tt = gates.tile([128, S], GD, tag="te", name="tt")
                nc.vector.tensor_mul(tt, zs[1][:], r)
                c = gates.tile([128, S], GD, tag="c", name="c")
                nc.scalar.activation(c, tt, mybir.ActivationFunctionType.Tanh)
                if t == 0:
                    nc.vector.tensor_mul(h_sb[m], u, c)
                else:
                    d = gates.tile([128, S], GD, tag="d", name="d")
                    nc.vector.tensor_sub(d, c, h_sb[m])
                    e = gates.tile([128, S], GD, tag="te2", name="e")
                    nc.vector.tensor_mul(e, u, d)
                    nc.vector.tensor_add(h_sb[m], h_sb[m], e)
                if mm == "bf16":
                    hb = gates.tile([128, S], BF16, tag="hb", name="hb")
                    nc.vector.tensor_copy(hb, h_sb[m])
                    hb_tiles[m] = hb
            tr_pend[m] = True

        tail_state = [None] * n_btiles
        for t in range(n_steps):
            for m in range(n_btiles):
                other = (m + 1) % n_btiles
                msl = slice(m * 128, (m + 1) * 128)
                nc.sync.dma_start(out=hT[m][0:2, KT, :], in_=d_at[:, t, msl])
                ps = []
                st6 = stats.tile([128, 2 * NG, 6], F32, tag="bnst", name="st6")
                if tr_pend[m]:
                    make_hT(m, hb_tiles[m])
                    tr_pend[m] = False
                for g in range(NG):
                    pt = psum.tile([128, 1024], F32, tag="parts", name="pt")
                    ps.append(pt)
                    for j in range(2):
                        nsl = slice(g * 1024 + j * 512, g * 1024 + (j + 1) * 512)
                        jsl = slice(j * 512, (j + 1) * 512)
                        kts = [KT] if t == 0 else [KT] + list(range(KT))
                        for i, k in enumerate(kts):
                            nc.tensor.matmul(
                                pt[:, jsl],
                                mmview(hT[m][:, k, :]),
                                mmview(wh_sb[:, k, nsl]),
                                start=(i == 0),
                                stop=(i == len(kts) - 1),
                            )
                        nc.vector.bn_stats(out=st6[:, 2 * g + j, :], in_=pt[:, jsl])
                tail_state[m] = (st6, stats.tile([128, 2], F32, tag="mv",
                                                 name="mv"), ps)
                emit_tail(m, t)

        # epilogue: h_last + decoder
        for m in range(n_btiles):
            msl = slice(m * 128, (m + 1) * 128)
            if tr_pend[m]:
                make_hT(m, hb_tiles[m])
                tr_pend[m] = False
            nc.sync.dma_start(out=d_h[msl, :], in_=h_sb[m])
            py = psum.tile([128, INP], F32, tag="parts", name="py")
            for k in range(KT):
                nc.tensor.matmul(
                    py,
                    mmview(hT[m][:, k, :]),
                    mmview(wd_sb[:, k, :]),
                    start=(k == 0),
                    stop=(k == KT - 1),
                )
            ysb = gates.tile([128, INP], F32, tag="y", name="ysb")
            nc.vector.tensor_add(ysb, py, bd_sb)
            nc.sync.dma_start(out=d_y[msl, :], in_=ysb)

    nc.compile()
    return nc


def _prep_inputs(action, W_emb, W_gru, ln_gamma, ln_beta, W_dec, b_dec,
                 fast_ln, variant=DEFAULT_VARIANT, n_steps=F, n_btiles=NBT):
    """Host-side weight folding + per-core shard dicts."""
    import ml_dtypes

    mm, _ = variant.split(":")
    bf = ml_dtypes.bfloat16
    mmdt = bf if mm == "bf16" else np.float32
    W_comb = (W_emb.astype(np.float64) @ W_gru[:S].astype(np.float64))
    wc = np.ascontiguousarray(W_comb.astype(np.float32).astype(mmdt))      # [2, 3S]
    wh = np.ascontiguousarray(
        W_gru[S:].astype(np.float32).reshape(KT, 128, S3).transpose(1, 0, 2)
        .astype(mmdt))                                                     # [128, KT, 3S]
    wd = np.ascontiguousarray(
        W_dec.astype(np.float32).reshape(KT, 128, INP).transpose(1, 0, 2)
        .astype(mmdt))                                                     # [128, KT, 2]
    bd = np.ascontiguousarray(
        np.broadcast_to(b_dec.astype(np.float32), (128, INP)))             # [128, 2]

    bl = n_btiles * 128
    in_maps = []
    for cix in range(NCORES):
        a_c = action[cix * BL: cix * BL + bl, :n_steps, :]                 # [bl, t, 2]
        at = np.ascontiguousarray(a_c.transpose(2, 1, 0).astype(mmdt))     # [2, t, bl]
        m = {"a_t": at, "w_h": wh, "w_c": wc, "w_d": wd, "b_d": bd}
        if not fast_ln:
            m["gm"] = np.ascontiguousarray(ln_gamma.astype(np.float32))[None, :]
            m["bt"] = np.ascontiguousarray(ln_beta.astype(np.float32))[None, :]
        in_maps.append(m)
    return in_maps


def kernel(action, W_emb, W_gru, ln_gamma, ln_beta, W_dec, b_dec):
    global LAST_RESULT
    action = np.asarray(action)
    W_emb = np.asarray(W_emb)
    W_gru = np.asarray(W_gru)
    ln_gamma = np.asarray(ln_gamma)
    ln_beta = np.asarray(ln_beta)
    W_dec = np.asarray(W_dec)
    b_dec = np.asarray(b_dec)

    fast_ln = bool(np.all(ln_gamma == 1.0) and np.all(ln_beta == 0.0))
    variant = DEFAULT_VARIANT
    key = ("nc", fast_ln, variant)
    if key not in _CACHE:
        _CACHE[key] = _build_nc(fast_ln, variant)
    nc = _CACHE[key]

    in_maps = _prep_inputs(
        action, W_emb, W_gru, ln_gamma, ln_beta, W_dec, b_dec, fast_ln, variant)

    trace = bool(os.environ.get("GRU_TRACE"))
    res = run_bass_kernel_spmd(
        nc, in_maps, core_ids=list(range(NCORES)), trace=trace)
    LAST_RESULT = res

    h_full = np.concatenate(
        [np.asarray(res.results[c]["h_out"]) for c in range(NCORES)], axis=0)
    y_full = np.concatenate(
        [np.asarray(res.results[c]["y_out"]) for c in range(NCORES)], axis=0)
    return h_full[:, None, :].astype(np.float32), y_full[:, None, :].astype(np.float32)


# revision 18
# speedup vs baseline: 1.1962x; 1.0009x over previous
"""Trainium2 Bass kernel for nn_Action_PureGRU.

Reference computation (see problem):
    emb = action @ W_emb                       # [B, F, S]
    per step t: parts = [x_t, h] @ W_gru       # [B, 3S]
                LN(parts) (gamma/beta), gates, h update
    h_last, outputs = h, h @ W_dec + b_dec

Strategy: data-parallel over batch across 8 NeuronCores (B=2048 -> 256/core,
processed as two 128-partition tiles). Weights replicated. The input path is
algebraically folded: action @ (W_emb @ W_gru[:S]) so each step's matmul is a
K=1026 contraction (8 k-tiles of h + 1 tiny K=2 tile of the raw action) done
on the PE with fp32 PSUM accumulation. LayerNorm statistics come from chunked
bn_stats on DVE (overlapped with the matmuls); the normalization is folded
into the ScalarE sigmoid/tanh activations via per-partition scale/bias.
1/sqrt(var+eps) is computed on DVE with the int32 bit-trick + 2 Newton steps
(avoids an ACT table swap: sqrt is not in the sigmoid/tanh table set).
h is kept fp32. For the next step's stationary matmul operand h must be
transposed (contraction dim on partitions):
  - mm="bf16": h cast to bf16, one DMA xbar block-transpose per batch tile.
  - mm="f32r": PE transpose (fp32, exact) + ScalarE copy PSUM->SBUF; matmul
    operands bitcast to float32r (full-rate at N>=256, higher precision
    than bf16).
"""

import os
from contextlib import ExitStack

import numpy as np

import concourse.bass as bass
import concourse.bacc as bacc
import concourse.tile as tile
from concourse import mybir
from concourse.bass_utils import run_bass_kernel_spmd
from concourse.masks import make_identity

# Problem constants (hardcoded per harness contract).
B, F, INP, S = 2048, 50, 2, 1024
S3 = 3 * S
LN_EPS = 1e-3
UPDATE_BIAS = -1.0
NCORES = 8
BL = B // NCORES          # 256 rows per core
NBT = BL // 128           # 2 batch tiles of 128 per core
KT = S // 128             # 8 k-tiles of h
NG = 3                    # three 1024-wide gate blocks
F32 = mybir.dt.float32
F32R = mybir.dt.float32r
BF16 = mybir.dt.bfloat16
I32 = mybir.dt.int32

# Variant: mm dtype for the recurrent matmul, dtype for gate tensors.
# f32r matmuls (full PE rate at N>=256, ~12-bit mantissa) + fp32 gates
# measured 1.0e-3 rel err over 50 steps on hardware.
DEFAULT_VARIANT = os.environ.get("GRU_VARIANT", "f32r:f32")

_CACHE: dict = {}
LAST_RESULT = None  # BassKernelResults of the most recent run (for test.py)


def _build_nc(fast_ln: bool, variant: str = DEFAULT_VARIANT,
              n_steps: int = F, n_btiles: int = NBT):
    """Build + compile the per-core Bass program (same program on all cores)."""
    mm, gd = variant.split(":")
    assert mm in ("bf16", "f32r") and gd in ("bf16", "f32")
    MMD = BF16 if mm == "bf16" else F32R  # storage dtype of W_h / W_dec / hT
    GD = BF16 if gd == "bf16" else F32    # gate tensor dtype

    def mmview(ap):  # matmul-operand view of an MMD tensor
        return ap

    nc = bacc.Bacc("TRN2", target_bir_lowering=False, debug=False)

    # ---- DRAM I/O (per core) ----
    bl = n_btiles * 128
    d_at = nc.dram_tensor("a_t", [2, n_steps, bl], MMD, kind="ExternalInput").ap()
    d_wh = nc.dram_tensor("w_h", [128, KT, S3], MMD, kind="ExternalInput").ap()
    d_wc = nc.dram_tensor("w_c", [2, S3], MMD, kind="ExternalInput").ap()
    d_wd = nc.dram_tensor("w_d", [128, KT, INP], MMD, kind="ExternalInput").ap()
    d_bd = nc.dram_tensor("b_d", [128, INP], F32, kind="ExternalInput").ap()
    if not fast_ln:
        d_gm = nc.dram_tensor("gm", [1, S3], F32, kind="ExternalInput").ap()
        d_bt = nc.dram_tensor("bt", [1, S3], F32, kind="ExternalInput").ap()
    d_h = nc.dram_tensor("h_out", [bl, S], F32, kind="ExternalOutput").ap()
    d_y = nc.dram_tensor("y_out", [bl, INP], F32, kind="ExternalOutput").ap()

    N_NEWTON = int(os.environ.get("GRU_NEWTON", "2"))
    MAGIC = 0x5F3759DF + 1  # rsqrt seed: C+1 (the +1 folds two's-complement negate)

    with tile.TileContext(nc) as tc, ExitStack() as ctx:
        singles = ctx.enter_context(tc.tile_pool(name="singles", bufs=1))
        psum = ctx.enter_context(tc.tile_pool(name="psum", bufs=4, space="PSUM"))
        gates = ctx.enter_context(tc.tile_pool(name="gates", bufs=2))
        stats = ctx.enter_context(tc.tile_pool(name="stats", bufs=4))

        # Resident weights / inputs.
        wh_sb = singles.tile([128, KT + 1, S3], MMD)
        nc.vector.memset(wh_sb[:, KT, :].bitcast(F32), 0.0)
        nc.sync.dma_start(out=wh_sb[:, :KT, :], in_=d_wh)
        nc.sync.dma_start(out=wh_sb[0:2, KT, :], in_=d_wc)
        wd_sb = singles.tile([128, KT, INP], MMD)
        nc.sync.dma_start(out=wd_sb, in_=d_wd)
        bd_sb = singles.tile([128, INP], F32)
        nc.sync.dma_start(out=bd_sb, in_=d_bd)
        if not fast_ln:
            gm_sb = singles.tile([128, S3], F32)
            nc.sync.dma_start(out=gm_sb, in_=d_gm.to_broadcast((128, S3)))
            bt_sb = singles.tile([128, S3], F32)
            nc.sync.dma_start(out=bt_sb, in_=d_bt.to_broadcast((128, S3)))
        if mm == "f32r":
            ident = singles.tile([128, 128], F32)
            make_identity(nc, ident)

        # Persistent per-btile state.
        h_sb = [singles.tile([128, S], F32, tag=f"h{m}", name=f"h{m}")
                for m in range(n_btiles)]
        hT = [singles.tile([128, KT + 1, 128], MMD, tag=f"hT{m}", name=f"hT{m}")
              for m in range(n_btiles)]
        for m in range(n_btiles):
            nc.vector.memset(hT[m][:, KT, :].bitcast(F32) if mm == "f32r"
                             else hT[m][:, KT, :], 0.0)
        hb_tiles = [None] * n_btiles

        def make_hT(m, hb):
            """hT[m][:, k, :] = h[:, k*128:(k+1)*128].T  (hb = bf16 copy or None)"""
            if mm == "bf16":
                eng = nc.sync if m % 2 == 0 else nc.scalar
                eng.dma_start_transpose(hT[m][:, :KT, :], hb)
            else:
                tp = psum.tile([128, 1024], F32, tag="parts", name="tp")
                for k in range(KT):
                    nc.tensor.transpose(
                        tp[:, k * 128:(k + 1) * 128], h_sb[m][:, k * 128:(k + 1) * 128],
                        ident,
                    )
                nc.scalar.copy(hT[m][:, :KT, :], tp)

        tr_pend = [False] * n_btiles

        def emit_tail(m, t):
            """LN stats + gates + state update for (t, m); halved passes."""
            st6, mv, ps = tail_state[m]
            nc.vector.bn_aggr(out=mv, in_=st6)
            vpe = stats.tile([128, 1], F32, tag="vpe", name="vpe")
            nc.vector.tensor_scalar_add(vpe, mv[:, 1:2], LN_EPS)
            yi = stats.tile([128, 1], I32, tag="yi", name="yi")
            nc.vector.tensor_scalar(
                out=yi,
                in0=vpe.bitcast(I32),
                scalar1=1,
                scalar2=-1,
                op0=mybir.AluOpType.logical_shift_right,
                op1=mybir.AluOpType.bitwise_xor,
            )
            nc.vector.tensor_scalar_add(yi, yi, MAGIC)
            y = yi.bitcast(F32)
            for it in range(N_NEWTON):
                t1 = stats.tile([128, 1], F32, tag="nt1", name="nt1")
                nc.vector.tensor_mul(t1, y, y)
                nc.vector.scalar_tensor_tensor(
                    out=t1, in0=t1, scalar=-0.5, in1=vpe,
                    op0=mybir.AluOpType.mult, op1=mybir.AluOpType.mult,
                )
                yn = stats.tile([128, 1], F32, tag="yn", name="yn")
                nc.vector.scalar_tensor_tensor(
                    out=yn, in0=t1, scalar=1.5, in1=y,
                    op0=mybir.AluOpType.add, op1=mybir.AluOpType.mult,
                )
                y = yn
            inv_std = y
            s2 = stats.tile([128, 1], F32, tag="s2", name="s2")
            nc.vector.scalar_tensor_tensor(
                out=s2, in0=mv[:, 0:1], scalar=-1.0, in1=inv_std,
                op0=mybir.AluOpType.mult, op1=mybir.AluOpType.mult,
            )
            s2u = stats.tile([128, 1], F32, tag="s2u", name="s2u")
            nc.vector.tensor_scalar_add(s2u, s2, UPDATE_BIAS)

            if fast_ln:
                zr, zc, zu = ps[0][:], ps[1][:], ps[2][:]
                r = gates.tile([128, S], GD, tag="rd", name="r")
                u = gates.tile([128, S], GD, tag="u", name="u")
                tt = gates.tile([128, S], GD, tag="te", name="tt")
                c = gates.tile([128, S], GD, tag="c", name="c")
                if t > 0:
                    d = gates.tile([128, S], GD, tag="d", name="d")
                    e = gates.tile([128, S], GD, tag="te2", name="e")
                if mm == "bf16":
                    hb = gates.tile([128, S], BF16, tag="hb", name="hb")
                    hb_tiles[m] = hb
                nc.scalar.activation(
                    r, zr, mybir.ActivationFunctionType.Sigmoid,
                    bias=s2, scale=inv_std,
                )
                nc.scalar.activation(
                    u, zu, mybir.ActivationFunctionType.Sigmoid,
                    bias=s2u, scale=inv_std,
                )
                nc.vector.scalar_tensor_tensor(
                    out=tt, in0=zc, scalar=mv[:, 0:1], in1=r,
                    op0=mybir.AluOpType.subtract, op1=mybir.AluOpType.mult,
                )
                nc.scalar.activation(
                    c, tt, mybir.ActivationFunctionType.Tanh, scale=inv_std,
                )
                if t == 0:
                    nc.vector.tensor_mul(h_sb[m], u, c)
                else:
                    nc.vector.tensor_sub(d, c, h_sb[m])
                    nc.vector.tensor_mul(e, u, d)
                    nc.vector.tensor_add(h_sb[m], h_sb[m], e)
                if mm == "bf16":
                    nc.vector.tensor_copy(hb, h_sb[m])
            else:
                zs = []
                for g in range(NG):
                    zg = gates.tile([128, 1024], F32, tag=f"z{g}", name="zg")
                    nc.vector.tensor_scalar(
                        out=zg, in0=ps[g][:], scalar1=mv[:, 0:1],
                        scalar2=inv_std,
                        op0=mybir.AluOpType.subtract,
                        op1=mybir.AluOpType.mult,
                    )
                    gsl = slice(g * 1024, (g + 1) * 1024)
                    nc.vector.tensor_mul(zg, zg, gm_sb[:, gsl])
                    nc.vector.tensor_add(zg, zg, bt_sb[:, gsl])
                    zs.append(zg)
                r = gates.tile([128, S], GD, tag="rd", name="r")
                nc.scalar.activation(
                    r, zs[0][:], mybir.ActivationFunctionType.Sigmoid)
                u = gates.tile([128, S], GD, tag="u", name="u")
                nc.scalar.activation(
                    u, zs[2][:], mybir.ActivationFunctionType.Sigmoid,
                    bias=UPDATE_BIAS, scale=1.0,
                )
                tt = gates.tile([128, S], GD, tag="te", name="tt")
                nc.vector.tensor_mul(tt, zs[1][:], r)
                c = gates.tile([128, S], GD, tag="c", name="c")
                nc.scalar.activation(c, tt, mybir.ActivationFunctionType.Tanh)
                if t == 0:
                    nc.vector.tensor_mul(h_sb[m], u, c)
                else:
                    d = gates.tile([128, S], GD, tag="d", name="d")
                    nc.vector.tensor_sub(d, c, h_sb[m])
                    e = gates.tile([128, S], GD, tag="te2", name="e")
                    nc.vector.tensor_mul(e, u, d)
                    nc.vector.tensor_add(h_sb[m], h_sb[m], e)
                if mm == "bf16":
                    hb = gates.tile([128, S], BF16, tag="hb", name="hb")
                    nc.vector.tensor_copy(hb, h_sb[m])
                    hb_tiles[m] = hb
            tr_pend[m] = True

        tail_state = [None] * n_btiles
        for t in range(n_steps):
            for m in range(n_btiles):
                other = (m + 1) % n_btiles
                msl = slice(m * 128, (m + 1) * 128)
                nc.sync.dma_start(out=hT[m][0:2, KT, :], in_=d_at[:, t, msl])
                ps = []
                st6 = stats.tile([128, 2 * NG, 6], F32, tag="bnst", name="st6")
                for g in range(NG):
                    pt = psum.tile([128, 1024], F32, tag="parts", name="pt")
                    ps.append(pt)
                    for j in range(2):
                        if g == NG - 1 and j == 1 and tr_pend[other]:
                            make_hT(other, hb_tiles[other])
                            tr_pend[other] = False
                        nsl = slice(g * 1024 + j * 512, g * 1024 + (j + 1) * 512)
                        jsl = slice(j * 512, (j + 1) * 512)
                        kts = [KT] if t == 0 else [KT] + list(range(KT))
                        for i, k in enumerate(kts):
                            nc.tensor.matmul(
                                pt[:, jsl],
                                mmview(hT[m][:, k, :]),
                                mmview(wh_sb[:, k, nsl]),
                                start=(i == 0),
                                stop=(i == len(kts) - 1),
                            )
                        nc.vector.bn_stats(out=st6[:, 2 * g + j, :], in_=pt[:, jsl])
                tail_state[m] = (st6, stats.tile([128, 2], F32, tag="mv",
                                                 name="mv"), ps)
                emit_tail(m, t)

        # epilogue: h_last + decoder
        for m in range(n_btiles):
            msl = slice(m * 128, (m + 1) * 128)
            if tr_pend[m]:
                make_hT(m, hb_tiles[m])
                tr_pend[m] = False
            nc.sync.dma_start(out=d_h[msl, :], in_=h_sb[m])
            py = psum.tile([128, INP], F32, tag="parts", name="py")
            for k in range(KT):
                nc.tensor.matmul(
                    py,
                    mmview(hT[m][:, k, :]),
                    mmview(wd_sb[:, k, :]),
                    start=(k == 0),
                    stop=(k == KT - 1),
                )
            ysb = gates.tile([128, INP], F32, tag="y", name="ysb")
            nc.vector.tensor_add(ysb, py, bd_sb)
            nc.sync.dma_start(out=d_y[msl, :], in_=ysb)

    nc.compile()
    return nc


def _prep_inputs(action, W_emb, W_gru, ln_gamma, ln_beta, W_dec, b_dec,
                 fast_ln, variant=DEFAULT_VARIANT, n_steps=F, n_btiles=NBT):
    """Host-side weight folding + per-core shard dicts."""
    import ml_dtypes

    mm, _ = variant.split(":")
    bf = ml_dtypes.bfloat16
    mmdt = bf if mm == "bf16" else np.float32
    W_comb = (W_emb.astype(np.float64) @ W_gru[:S].astype(np.float64))
    wc = np.ascontiguousarray(W_comb.astype(np.float32).astype(mmdt))      # [2, 3S]
    wh = np.ascontiguousarray(
        W_gru[S:].astype(np.float32).reshape(KT, 128, S3).transpose(1, 0, 2)
        .astype(mmdt))                                                     # [128, KT, 3S]
    wd = np.ascontiguousarray(
        W_dec.astype(np.float32).reshape(KT, 128, INP).transpose(1, 0, 2)
        .astype(mmdt))                                                     # [128, KT, 2]
    bd = np.ascontiguousarray(
        np.broadcast_to(b_dec.astype(np.float32), (128, INP)))             # [128, 2]

    bl = n_btiles * 128
    in_maps = []
    for cix in range(NCORES):
        a_c = action[cix * BL: cix * BL + bl, :n_steps, :]                 # [bl, t, 2]
        at = np.ascontiguousarray(a_c.transpose(2, 1, 0).astype(mmdt))     # [2, t, bl]
        m = {"a_t": at, "w_h": wh, "w_c": wc, "w_d": wd, "b_d": bd}
        if not fast_ln:
            m["gm"] = np.ascontiguousarray(ln_gamma.astype(np.float32))[None, :]
            m["bt"] = np.ascontiguousarray(ln_beta.astype(np.float32))[None, :]
        in_maps.append(m)
    return in_maps


def kernel(action, W_emb, W_gru, ln_gamma, ln_beta, W_dec, b_dec):
    global LAST_RESULT
    action = np.asarray(action)
    W_emb = np.asarray(W_emb)
    W_gru = np.asarray(W_gru)
    ln_gamma = np.asarray(ln_gamma)
    ln_beta = np.asarray(ln_beta)
    W_dec = np.asarray(W_dec)
    b_dec = np.asarray(b_dec)

    fast_ln = bool(np.all(ln_gamma == 1.0) and np.all(ln_beta == 0.0))
    variant = DEFAULT_VARIANT
    key = ("nc", fast_ln, variant)
    if key not in _CACHE:
        _CACHE[key] = _build_nc(fast_ln, variant)
    nc = _CACHE[key]

    in_maps = _prep_inputs(
        action, W_emb, W_gru, ln_gamma, ln_beta, W_dec, b_dec, fast_ln, variant)

    trace = bool(os.environ.get("GRU_TRACE"))
    res = run_bass_kernel_spmd(
        nc, in_maps, core_ids=list(range(NCORES)), trace=trace)
    LAST_RESULT = res

    h_full = np.concatenate(
        [np.asarray(res.results[c]["h_out"]) for c in range(NCORES)], axis=0)
    y_full = np.concatenate(
        [np.asarray(res.results[c]["y_out"]) for c in range(NCORES)], axis=0)
    return h_full[:, None, :].astype(np.float32), y_full[:, None, :].astype(np.float32)
